# revision 40
# baseline (speedup 1.0000x reference)
"""Associative-embedding (push/pull) loss on 8 TRN2 NeuronCores.

Strategy (data parallel, 8 images per core, balanced):
  - The 285MB tags tensor is only touched at P*K=510 points per image, and
    only ~1/4 of those are valid. Images are BIN-PACKED onto cores so every
    core's valid-point count fits exactly C=8 indirect-DMA windows of 128
    single-element descriptors (the HW contract: one index per destination
    partition). Larger counts fall back to a lazily compiled wider variant.
  - Per 128-point block, one fp16 PE matmul scatter-accumulates the moment
    pair (v, v^2) into a PSUM tile s12E[120, 4] laid out as
    (rho=(g,p), (moment, b)) with img = 4*b + g. The one-hot point->rho
    selection matrix and the img_hi (b) mask are host-built uploads;
    on-device per block: a square and two tiny masked mults (f16 out).
  - Pull and push collapse via two small diagonal matmuls (ic3^T dd and
    c1R^T sacc); the host sums the [2,4] diagonals and subtracts the
    constant c2 term.
  - Push uses an invalid-person fake-mean offset (no pair mask) in a
    [120, 60] layout: a host-built group-select fp16 matmul replicates the
    per-image mean row into all 30 partition rows of its image group, so
    the pairwise difference, Square, and Exp all run 120 partitions wide.
  - Each core emits (push, pull) partials already scaled by 1/64; the host
    sums the 8 partials.
"""

import sys

import numpy as np

if "/opt/trn_rl_repo" not in sys.path:
    sys.path.insert(0, "/opt/trn_rl_repo")

from concourse import bacc, bass, mybir  # noqa: E402
from concourse import bass_utils  # noqa: E402

B, P, K, H, W = 64, 30, 17, 256, 256
NCORES = 8
BPC = B // NCORES           # 8 images per core
NTOT = BPC * K * H * W      # flat tag elements per core
C_FAST = 8                  # 1024-point capacity (valid ~1019 after balance)

R = 120                     # rho = g*30 + p, g = img%4, p = person
PSW = 120                   # per-call pselh cols: 120 one-hot
AUXF_W = 8                  # invE 0:2, fakeE 2:4, ic3E 4:6, c1R 6:8
AUXH_W = 210                # IDPh 0:30, GSELh 30:150, fakeIDP 150:210

f32 = mybir.dt.float32
f16 = mybir.dt.float16
i32 = mybir.dt.int32
Alu = mybir.AluOpType
Act = mybir.ActivationFunctionType
AX = mybir.AxisListType


def build_nc_raw(ncalls=C_FAST):
    """Raw Block-mode build: manual semaphores, no Tile scaffolding."""
    from contextlib import ExitStack

    nc = bacc.Bacc("TRN2", target_bir_lowering=False, debug=False,
                   num_devices=NCORES)

    tags = nc.dram_tensor("tags", [NTOT, 1], f32, kind="ExternalInput")
    idx_in = nc.dram_tensor("idx", [128, ncalls], i32, kind="ExternalInput")
    psel_in = nc.dram_tensor("pselh", [128, PSW * ncalls], f16,
                             kind="ExternalInput")
    bmf_in = nc.dram_tensor("bmf", [128, 2 * ncalls], f32,
                            kind="ExternalInput")
    auxf_in = nc.dram_tensor("auxf", [R, AUXF_W], f32, kind="ExternalInput")
    auxh_in = nc.dram_tensor("auxh", [R, AUXH_W], f16, kind="ExternalInput")
    out = nc.dram_tensor("out", [2, 4], f32, kind="ExternalOutput")

    with ExitStack() as ctx:
        block = ctx.enter_context(nc.Block())
        sb = lambda n, s, d: ctx.enter_context(nc.sbuf_tensor(n, s, d))  # noqa: E731
        ps = lambda n, s: ctx.enter_context(nc.psum_tensor(n, s, f32))  # noqa: E731
        sem = lambda n: ctx.enter_context(nc.semaphore(n))  # noqa: E731

        idxm = sb("idxm", [128, ncalls], i32)
        pselh = sb("pselh_s", [128, PSW * ncalls], f16)
        bmf = sb("bmf_s", [128, ncalls, 2], f32)
        auxf = sb("auxf_s", [R, AUXF_W], f32)
        auxh = sb("auxh_s", [R, AUXH_W], f16)
        v4 = sb("v4", [128, ncalls, 1], f32)
        q2 = sb("q2", [128, ncalls, 1], f32)
        vvb = sb("vvb", [128, ncalls, 2, 2], f16)
        meanE = sb("meanE", [R, 2], f32)
        sm = sb("sm", [R, 2], f32)
        dd = sb("dd", [R, 2], f32)
        meanF16 = sb("meanF16", [R, 2], f16)
        meanFr = sb("meanFr", [R, 2], f32)
        rhsm = sb("rhsm", [R, 2, 30], f16)
        dmt = sb("dmt", [R, 2, 30], f32)
        sq = sb("sq", [R, 2, 30], f32)
        e = sb("e", [R, 2, 30], f32)
        sacc = sb("sacc", [R, 2], f32)
        res = sb("res", [2, 4], f32)
        scr2 = sb("scr2", [1, 8], f32)

        s12a = ps("s12a", [R, 2])
        s12b = ps("s12b", [R, 2])
        mrep = ps("mrep", [R, 2, 30])
        finp = ps("finp", [2, 4])

        s_idx = sem("s_idx")
        s_bm = sem("s_bm")
        s_psel = sem("s_psel")
        s_auxf = sem("s_auxf")
        s_auxh = sem("s_auxh")
        s_out = sem("s_out")
        gsems = [sem(f"gsem{c}") for c in range(ncalls)]
        vchain = sem("vchain")
        vsem = sem("vsem")
        s12sem = sem("s12sem")
        s1sem = sem("s1sem")
        rhsem = sem("rhsem")
        mrepsem = sem("mrepsem")
        sqsem = sem("sqsem")
        esem = sem("esem")
        ddsem = sem("ddsem")
        saccsem = sem("saccsem")
        finsem = sem("finsem")
        ressem = sem("ressem")

        @block.sync
        def _(sy):
            sy.dma_start(out=bmf[:], in_=bmf_in[:]).then_inc(s_bm, 16)
            sy.dma_start(out=pselh[:], in_=psel_in[:]).then_inc(s_psel, 16)
            sy.dma_start(out=auxf[:], in_=auxf_in[:]).then_inc(s_auxf, 16)
            sy.dma_start(out=auxh[:], in_=auxh_in[:]).then_inc(s_auxh, 16)
            sy.wait_ge(ressem, 1)
            sy.dma_start(out=out[:], in_=res[:]).then_inc(s_out, 16)
            sy.wait_ge(s_out, 16)

        @block.gpsimd
        def _(g):
            g.wait_ge(s_idx, 16)
            for c in range(ncalls):
                g.indirect_dma_start(
                    out=v4[:, c, :], out_offset=None, in_=tags[:],
                    in_offset=bass.IndirectOffsetOnAxis(
                        ap=idxm[:, c:c + 1], axis=0)).then_inc(gsems[c], 16)

        @block.vector
        def _(v):
            vch = [0]

            def inc(inst):
                inst.then_inc(vchain, 1)
                vch[0] += 1

            def wait():
                v.wait_ge(vchain, vch[0])

            v.wait_ge(s_bm, 16)
            for c in range(ncalls):
                v.wait_ge(gsems[c], 16)
                vc = v4[:, c, :]
                last = c == ncalls - 1
                if last:
                    # vvb0 first: unblocks the s1-half matmul immediately
                    v.tensor_tensor(out=vvb[:, c, 0, :],
                                    in0=vc.to_broadcast([128, 2]),
                                    in1=bmf[:, c, :],
                                    op=Alu.mult).then_inc(vsem, 1)
                    inc(v.tensor_tensor(out=q2[:, c, :], in0=vc, in1=vc,
                                        op=Alu.mult))
                    wait()
                    v.tensor_tensor(out=vvb[:, c, 1, :],
                                    in0=q2[:, c, :].to_broadcast([128, 2]),
                                    in1=bmf[:, c, :],
                                    op=Alu.mult).then_inc(vsem, 1)
                else:
                    inc(v.tensor_tensor(out=q2[:, c, :], in0=vc, in1=vc,
                                        op=Alu.mult))
                    v.tensor_tensor(out=vvb[:, c, 0, :],
                                    in0=vc.to_broadcast([128, 2]),
                                    in1=bmf[:, c, :], op=Alu.mult)
                    wait()
                    v.tensor_tensor(out=vvb[:, c, 1, :],
                                    in0=q2[:, c, :].to_broadcast([128, 2]),
                                    in1=bmf[:, c, :],
                                    op=Alu.mult).then_inc(vsem, 1)
            v.wait_ge(s_auxf, 16)
            v.wait_ge(s_auxh, 16)
            v.wait_ge(s1sem, 1)
            inc(v.tensor_tensor(out=meanF16[:], in0=s12a[:],
                                in1=auxf[:, 0:2], op=Alu.mult))
            wait()
            v.tensor_tensor(
                out=rhsm[:],
                in0=meanF16[:].unsqueeze(2).to_broadcast([R, 2, 30]),
                in1=auxh[:, 0:30].unsqueeze(1).to_broadcast([R, 2, 30]),
                op=Alu.mult).then_inc(rhsem, 1)
            # shadowed: f32 mean, pull ops, meanFr (mean + f16-exact fake)
            inc(v.tensor_copy(out=meanE[:], in_=meanF16[:]))
            wait()
            inc(v.tensor_tensor(out=meanFr[:], in0=meanE[:],
                                in1=auxf[:, 2:4], op=Alu.add))
            inc(v.tensor_tensor(out=sm[:], in0=s12a[:], in1=meanE[:],
                                op=Alu.mult))
            wait()
            v.wait_ge(s12sem, 1)
            v.tensor_tensor(out=dd[:], in0=s12b[:], in1=sm[:],
                            op=Alu.subtract).then_inc(ddsem, 1)
            v.wait_ge(mrepsem, 1)
            inc(v.tensor_tensor(
                out=dmt[:],
                in0=meanFr[:].unsqueeze(2).to_broadcast([R, 2, 30]),
                in1=mrep[:], op=Alu.subtract))
            wait()
            v.tensor_tensor(out=sq[:], in0=dmt[:], in1=dmt[:],
                            op=Alu.mult).then_inc(sqsem, 1)
            v.wait_ge(esem, 1)
            v.tensor_reduce(out=sacc[:], in_=e[:], axis=AX.X,
                            op=Alu.add).then_inc(saccsem, 1)
            v.wait_ge(finsem, 1)
            v.tensor_copy(out=res[:], in_=finp[:]).then_inc(ressem, 1)

        @block.tensor
        def _(t):
            t.wait_ge(s_auxh, 16)
            t.matmul(out=mrep[:], lhsT=auxh[:, 30:150],
                     rhs=auxh[:, 150:210], start=True, stop=False)
            t.wait_ge(s_psel, 16)
            for c in range(ncalls - 1):
                t.wait_ge(vsem, c + 1)
                t.matmul(out=s12a[:],
                         lhsT=pselh[:, PSW * c:PSW * c + 120],
                         rhs=vvb[:, c, 0, :], start=(c == 0), stop=False)
                t.matmul(out=s12b[:],
                         lhsT=pselh[:, PSW * c:PSW * c + 120],
                         rhs=vvb[:, c, 1, :], start=(c == 0), stop=False)
            cl = ncalls - 1
            t.wait_ge(vsem, cl + 1)
            t.matmul(out=s12a[:],
                     lhsT=pselh[:, PSW * cl:PSW * cl + 120],
                     rhs=vvb[:, cl, 0, :], start=False,
                     stop=True).then_inc(s1sem, 1)
            t.wait_ge(vsem, cl + 2)
            t.matmul(out=s12b[:],
                     lhsT=pselh[:, PSW * cl:PSW * cl + 120],
                     rhs=vvb[:, cl, 1, :], start=False,
                     stop=True).then_inc(s12sem, 1)
            t.wait_ge(rhsem, 1)
            t.matmul(out=mrep[:], lhsT=auxh[:, 30:150], rhs=rhsm[:],
                     start=False, stop=True).then_inc(mrepsem, 1)
            t.wait_ge(ddsem, 1)
            t.matmul(out=finp[:, 0:2], lhsT=auxf[:, 4:6], rhs=dd[:],
                     start=True, stop=True)
            t.wait_ge(saccsem, 1)
            t.matmul(out=finp[:, 2:4], lhsT=auxf[:, 6:8], rhs=sacc[:],
                     start=True, stop=True).then_inc(finsem, 1)

        @block.scalar
        def _(s):
            s.dma_start(out=idxm[:], in_=idx_in[:]).then_inc(s_idx, 16)
            # dummy Exp: hoists the ACT table load off the critical path
            s.activation(out=scr2[:],
                         in_=nc.const_aps.tensor(0.0, [1, 8], f32),
                         func=Act.Exp, scale=-1.0)
            s.wait_ge(sqsem, 1)
            s.activation(out=e[:], in_=sq[:], func=Act.Exp,
                         scale=-1.0).then_inc(esem, 1)

    nc.compile()
    return nc


_nc_cache = {}


def _get_nc(ncalls=C_FAST):
    if ncalls not in _nc_cache:
        _nc_cache[ncalls] = build_nc_raw(ncalls)
    return _nc_cache[ncalls]


def _balance_images(per_img):
    """LPT + swap refinement: 8 bins x 8 images, minimize max point total."""
    order = np.argsort(-per_img)
    bins = [[] for _ in range(NCORES)]
    tot = [0] * NCORES
    for i in order:
        cands = [b for b in range(NCORES) if len(bins[b]) < BPC]
        b = min(cands, key=lambda x: tot[x])
        bins[b].append(int(i))
        tot[b] += int(per_img[i])
    for _ in range(1000):
        hi = int(np.argmax(tot))
        best = None
        for lo in range(NCORES):
            if lo == hi:
                continue
            for ii, a in enumerate(bins[hi]):
                for jj, b2 in enumerate(bins[lo]):
                    delta = int(per_img[a]) - int(per_img[b2])
                    if delta > 0 and max(tot[hi] - delta,
                                         tot[lo] + delta) < tot[hi]:
                        best = (lo, ii, jj, delta)
                        break
                if best:
                    break
            if best:
                break
        if not best:
            break
        lo, ii, jj, delta = best
        bins[hi][ii], bins[lo][jj] = bins[lo][jj], bins[hi][ii]
        tot[hi] -= delta
        tot[lo] += delta
    return bins, max(tot)


def make_in_maps(tags, joints, jv, pv, ncalls=None):
    """Host preprocessing: per-core input dict. Returns (in_maps, ncalls)."""
    tags = np.asarray(tags, dtype=np.float32).reshape(B, K * H * W)
    joints = np.asarray(joints, dtype=np.int64)
    jv = np.asarray(jv)
    pv = np.asarray(pv)

    m_all = (jv > 0) & (pv[:, :, None] > 0)            # [64, 30, 17]
    bins, mx = _balance_images(m_all.sum((1, 2)))
    if ncalls is None:
        ncalls = max(C_FAST, -(-mx // 128))

    x_all = joints[:, :, :, 0]
    y_all = joints[:, :, :, 1]

    in_maps = []
    for core in range(NCORES):
        imgs = bins[core]
        m = m_all[imgs]                                 # [8, 30, 17]
        x = x_all[imgs]
        y = y_all[imgs]
        li_i, p_i, k_i = np.nonzero(m)
        addr = (65536 * (li_i * K + k_i) + 256 * x[li_i, p_i, k_i]
                + y[li_i, p_i, k_i]).astype(np.int64)
        order = np.argsort(addr, kind="stable")
        addr, li_i, p_i = addr[order], li_i[order], p_i[order]
        nv = addr.shape[0]
        assert nv <= 128 * ncalls, (nv, ncalls)

        t = np.arange(nv)
        q_t, c_t = t % 128, t // 128
        idxm = np.zeros((128, ncalls), dtype=np.int32)
        idxm[q_t, c_t] = addr
        g_i = li_i % 4
        b_i = li_i // 4
        pselh = np.zeros((128, PSW * ncalls), dtype=np.float16)
        pselh[q_t, PSW * c_t + g_i * 30 + p_i] = 1.0
        bmf = np.zeros((128, 2 * ncalls), dtype=np.float32)
        bmf[q_t, 2 * c_t + b_i] = 1.0

        cnt = m.sum(axis=2).astype(np.float32)          # [8 img, 30 p]
        n = (cnt > 0).sum(axis=1).astype(np.float32)    # [8]
        ninv = 1.0 / np.maximum(n, 1.0)
        den = np.maximum(n * (n - 1.0), 1.0)
        c1 = 0.5 * (n > 1) / den / B

        rho_g = np.arange(R) // 30                      # g
        rho_p = np.arange(R) % 30                       # p
        bb = np.arange(2)
        img_rb = rho_g[:, None] + 4 * bb[None, :]       # [120, 2] img idx
        cnt_rb = cnt[img_rb, rho_p[:, None]]            # [120, 2]
        auxf = np.zeros((R, AUXF_W), dtype=np.float32)
        auxf[:, 0:2] = 1.0 / np.maximum(cnt_rb, 1.0)
        auxf[:, 2:4] = (cnt_rb <= 0) * (1000.0 * (rho_p[:, None] + 1.0))
        auxf[:, 4:6] = auxf[:, 0:2] * (ninv[img_rb] / B)
        auxf[:, 6:8] = c1[img_rb]                       # c1R
        c2sum = float(P * c1.sum())

        fake16 = ((cnt_rb <= 0)
                  * (1000.0 * (rho_p[:, None] + 1.0))).astype(np.float16)
        auxf[:, 2:4] = fake16.astype(np.float32)
        idp = (rho_p[:, None] == np.arange(30)[None, :]).astype(np.float16)
        auxh = np.zeros((R, AUXH_W), dtype=np.float16)
        auxh[:, 0:30] = idp
        auxh[:, 30:150] = (rho_g[:, None] == rho_g[None, :])
        auxh[:, 150:210] = (fake16[:, :, None]
                            * idp[:, None, :]).reshape(R, 60)

        in_maps.append({
            "tags": np.ascontiguousarray(tags[imgs]).reshape(NTOT, 1),
            "idx": idxm,
            "pselh": pselh,
            "bmf": bmf,
            "auxf": auxf,
            "auxh": auxh,
            "_c2sum": c2sum,
        })
    return in_maps, ncalls


def kernel(tags, joints, joint_img_valid, person_valid):
    in_maps, ncalls = make_in_maps(tags, joints, joint_img_valid,
                                   person_valid)
    c2sums = [im.pop("_c2sum") for im in in_maps]
    nc = _get_nc(ncalls)
    res = bass_utils.run_bass_kernel_spmd(nc, in_maps,
                                          core_ids=list(range(NCORES)))
    push = pull = 0.0
    for r, c2s in zip(res.results, c2sums):
        o = np.asarray(r["out"], dtype=np.float64).reshape(2, 4)
        pull += o[0, 0] + o[1, 1]
        push += o[0, 2] + o[1, 3] - c2s
    return np.float32(push), np.float32(pull)


if __name__ == "__main__":
    rng = np.random.default_rng(0)
    t = rng.standard_normal((B, K, H, W), dtype=np.float32)
    j = rng.integers(0, H, size=(B, P, K, 2), dtype=np.int32)
    jv_ = rng.integers(0, 2, size=(B, P, K), dtype=np.int32)
    pv_ = rng.integers(0, 2, size=(B, P), dtype=np.int32)
    print(kernel(t, j, jv_, pv_))
